# revision 8
# baseline (speedup 1.0000x reference)
"""2-layer GCN (GCNConv 128->128->64, N=50000, E=800000) on 8 TRN2 NeuronCores.

Strategy (dst-sharded, aggregate-first, v2):
  out = relu(A_hat @ relu(A_hat @ x @ W1 + b1) @ W2 + b2),  A_hat = D^-1/2 (A+I) D^-1/2
  - Gather tables hold dis-scaled features (t1 = dis*x, t2 = dis*relu(...)), so
    one-hot scatter matrices are PURE 0/1 (single DVE is_equal per block-section).
  - dst-side dis folded into the per-block epilogue: weight matmul emitted as
    matmul(lhsT=t_sb, rhs=W) giving [dst, oc] output directly (no transposes);
    activation applies per-partition scale (dis or dis^2) + Relu; bias enters
    via a rank-1 (K=1) PSUM matmul outer(1/dis, b) before the scale.
  - Self-loop term is matmul(lhsT=stage_block, rhs=I) (transposes the block).
  - Edges sorted by (core, superblock of 7 dst-blocks, src-table-half, dst) and
    packed: ONE jumbo dma_gather per (superblock, half) (~5-10k idxs) instead
    of ~100 1k-idx calls -> tiny GPSIMD/SWDGE fixed overhead. Per-block tile
    subranges (uniform across cores) drive the PSUM-accumulated matmuls; edges
    of neighboring blocks inside shared boundary tiles are nulled by the
    one-hot (esc values are block-shifted on host so only own-block edges land
    in 0..127).
  - One AllGather per layer of the bf16 feature table (intra-chip, Shared out).
Host-side work is index-only prep + output concat.
"""

import numpy as np
import ml_dtypes

import concourse.bass as bass
import concourse.bacc as bacc
import concourse.mybir as mybir
import concourse.tile as tile
from concourse.bass_utils import run_bass_kernel_spmd
from concourse.library_config import mlp
from concourse.masks import make_identity

P = 128
N_NODES = 50000
N_EDGES = 800000
IN_CH = 128
HID_CH = 128
OUT_CH = 64
N_CORES = 8
NSH = N_NODES // N_CORES          # 6250 nodes per core
NBLK = (NSH + P - 1) // P         # 49 blocks per core (48 full + 106 tail)
SBW = 7                           # blocks per superblock
NSB = NBLK // SBW                 # 7 superblocks
VLO = 32768                       # low table half (int16 index range)
NWRAP = NBLK * P                  # 6272 table slots per core-shard
VTAB = N_CORES * P * NBLK         # 50176 table rows in [*, 128] view
NFULL = NSH // P                  # 48 full blocks
NTAIL = NSH - NFULL * P           # 106

BF16 = mybir.dt.bfloat16
F16 = mybir.dt.float16
F32 = mybir.dt.float32

LAST_RESULT = None  # for test harness: BassKernelResults of last run


def _host_prep(edge_index):
    """Index-only preprocessing. Returns per-core upload arrays + tile plan."""
    src = edge_index[0].astype(np.int64)
    dst = edge_index[1].astype(np.int64)
    E = src.shape[0]

    deg = np.bincount(dst, minlength=N_NODES) + 1
    dis = 1.0 / np.sqrt(deg.astype(np.float64))
    idis = np.sqrt(deg.astype(np.float64))

    core = dst // NSH
    ic = dst - core * NSH
    blk = ic // P
    sb = blk // SBW
    bi = blk - sb * SBW
    drel = ic - sb * (SBW * P)          # 0..895 superblock-relative dst

    # wrapped-padded table row of node v: (v//NSH)*... see _build table view
    k = src // NSH
    i = src - k * NSH
    srow = (k * P + (i % P)) * NBLK + i // P
    half = (srow >= VLO).astype(np.int64)

    skey = (core * NSB + sb) * 2 + half          # per-core section id
    order = np.lexsort((dst, skey))              # sort by section, then dst
    s_skey = skey[order]
    s_core = core[order]
    s_drel = drel[order]
    s_row = (srow - half * VLO)[order]

    n_sec = N_CORES * NSB * 2
    cnt = np.bincount(s_skey, minlength=n_sec).reshape(N_CORES, NSB * 2)
    T_sec = np.ceil(cnt.max(axis=0) / P).astype(np.int64)   # [NSB*2]
    TB = np.concatenate([[0], np.cumsum(T_sec)])[:-1]       # [NSB*2]
    T_total = int(T_sec.sum())

    sec_start = np.concatenate([[0], np.cumsum(cnt.reshape(-1))])[:-1]
    pos = np.arange(E) - sec_start[s_skey]
    slot = TB[s_skey % (NSB * 2)] * P + pos      # slot within core's edge list

    EPC = T_total * P
    idx_rows = np.zeros((N_CORES, EPC), np.int64)
    drel_arr = np.full((N_CORES, EPC), -3000, np.int64)
    idx_rows[s_core, slot] = s_row
    drel_arr[s_core, slot] = s_drel

    # per (sb, h, bi) tile subranges, uniform across cores
    key4 = skey * SBW + bi
    cnt_blk = np.bincount(key4, minlength=n_sec * SBW).reshape(
        N_CORES, NSB, 2, SBW)
    s0 = np.cumsum(cnt_blk, axis=3) - cnt_blk            # exclusive start
    e0 = s0 + cnt_blk
    ft_c = s0 // P
    lt_c = -(-e0 // P)
    has = cnt_blk > 0
    BIG = 10 ** 6
    ft = np.where(has, ft_c, BIG).min(axis=0)            # [NSB, 2, SBW]
    lt = np.where(has, lt_c, -1).max(axis=0)
    nt = np.maximum(lt - ft, 0)
    ft = np.where(nt > 0, ft, 0)
    MAXNT = int(nt.max())

    # esc_blk: per-(sb,h,bi) block-shifted dst-rel values, f16-exact ints
    drel_wrap = drel_arr.reshape(N_CORES, T_total, P).transpose(0, 2, 1)
    cols = []
    ebase = np.zeros((NSB, 2, SBW), np.int64)
    run = 0
    for sbx in range(NSB):
        for h in (0, 1):
            for bix in range(SBW):
                n = int(nt[sbx, h, bix])
                if n == 0:
                    ebase[sbx, h, bix] = -1
                    continue
                t0 = int(TB[sbx * 2 + h] + ft[sbx, h, bix])
                c = drel_wrap[:, :, t0:t0 + n] - 128 * bix
                cols.append(np.clip(c, -2000, 2000))
                ebase[sbx, h, bix] = run
                run += n
    escb = np.concatenate(cols, axis=2).astype(np.float16)  # [NC, P, NTT]
    NTT = run

    # wrap indices: idx i -> [i%16, i//16], replicated to 128 partitions
    ii = np.arange(EPC)
    w = np.zeros((N_CORES, 16, T_total * 8), np.int16)
    w[:, ii % 16, ii // 16] = idx_rows
    idxw = np.tile(w, (1, 8, 1))                 # [NC, 128, T*8]

    # disw / disw2 [NC, P, NBLK]; idisw [NC, 1, NBLK*P]
    nodes = np.arange(NBLK * P)
    valid = nodes < NSH
    disw = np.zeros((N_CORES, P, NBLK), np.float32)
    idisw = np.zeros((N_CORES, 1, NBLK * P), np.float32)
    for c in range(N_CORES):
        v = np.zeros(NBLK * P)
        v[valid] = dis[c * NSH + nodes[valid]]
        disw[c] = v.reshape(NBLK, P).T.astype(np.float32)
        u = np.zeros(NBLK * P)
        u[valid] = idis[c * NSH + nodes[valid]]
        idisw[c, 0] = u.astype(np.float32)
    disw2 = (disw.astype(np.float64) ** 2).astype(np.float32)

    iotar = np.tile(np.arange(P, dtype=np.float16), (P, max(MAXNT, 1), 1))

    return {
        "T_sec": T_sec.reshape(NSB, 2), "TB": TB.reshape(NSB, 2),
        "ft": ft, "nt": nt, "ebase": ebase, "T_total": T_total,
        "MAXNT": MAXNT, "NTT": NTT,
        "idxw": idxw.astype(np.int16), "escb": escb,
        "disw": disw, "disw2": disw2, "idisw": idisw, "iotar": iotar,
    }


def _chunks(t, cap):
    """Split t tiles into balanced chunks of <= cap (9 -> 5+4, not 8+1)."""
    if t == 0:
        return []
    if cap is None or t <= cap:
        return [t]
    n = -(-t // cap)
    base, rem = divmod(t, n)
    return [base + (1 if i < rem else 0) for i in range(n)]


def _build(prep, repeats=1, shared_tables=False, gcap=8, no_bias=False):
    T_sec, TB = prep["T_sec"], prep["TB"]
    ft, nt, ebase = prep["ft"], prep["nt"], prep["ebase"]
    T_total, MAXNT, NTT = prep["T_total"], prep["MAXNT"], prep["NTT"]
    TMAX = [int(T_sec[:, 0].max()), int(T_sec[:, 1].max())]

    nc = bacc.Bacc("TRN2", target_bir_lowering=False, num_devices=N_CORES,
                   num_swdge_queues=4)

    t_xsh = nc.dram_tensor("xsh", [NSH, IN_CH], F32, kind="ExternalInput")
    t_w1 = nc.dram_tensor("w1", [IN_CH, HID_CH], F32, kind="ExternalInput")
    t_b1 = nc.dram_tensor("b1", [1, HID_CH], F32, kind="ExternalInput")
    t_w2 = nc.dram_tensor("w2", [HID_CH, OUT_CH], F32, kind="ExternalInput")
    t_b2 = nc.dram_tensor("b2", [1, OUT_CH], F32, kind="ExternalInput")
    t_disw = nc.dram_tensor("disw", [P, NBLK], F32, kind="ExternalInput")
    t_disw2 = nc.dram_tensor("disw2", [P, NBLK], F32, kind="ExternalInput")
    t_idisw = nc.dram_tensor("idisw", [1, NBLK * P], F32, kind="ExternalInput")
    t_idxw = nc.dram_tensor("idxw", [P, T_total * 8], mybir.dt.int16,
                            kind="ExternalInput")
    t_escb = nc.dram_tensor("escb", [P, NTT], F16, kind="ExternalInput")
    t_iotar = nc.dram_tensor("iotar", [P, max(MAXNT, 1) * P], F16,
                             kind="ExternalInput")
    t_out = nc.dram_tensor("out", [NSH, OUT_CH], F32, kind="ExternalOutput")

    aspace = "Shared" if shared_tables else "Local"
    x1_shard = nc.dram_tensor("x1_shard", [P, NWRAP], BF16)
    x1_full = nc.dram_tensor("x1_full", [N_CORES * P, NWRAP], BF16,
                             addr_space=aspace)
    x2_shard = nc.dram_tensor("x2_shard", [P, NWRAP], BF16)
    x2_full = nc.dram_tensor("x2_full", [N_CORES * P, NWRAP], BF16,
                             addr_space=aspace)

    rg = [list(range(N_CORES))]

    with tile.TileContext(nc) as tc:
        with (
            tc.tile_pool(name="const", bufs=1) as cp,
            tc.tile_pool(name="gpool", bufs=2) as gp,
            tc.tile_pool(name="opool", bufs=4) as op,
            tc.tile_pool(name="sbuf", bufs=3) as sbp,
            tc.tile_pool(name="psum", bufs=2, space="PSUM") as ps,
        ):
            nc.gpsimd.load_library(mlp)

            idx_sb = cp.tile([P, T_total * 8], mybir.dt.int16)
            nc.sync.dma_start(out=idx_sb[:], in_=t_idxw[:, :])
            escb_sb = cp.tile([P, NTT], F16)
            nc.sync.dma_start(out=escb_sb[:], in_=t_escb[:, :])
            iota_sb = cp.tile([P, max(MAXNT, 1), P], F16)
            nc.sync.dma_start(
                out=iota_sb[:],
                in_=t_iotar[:, :].rearrange("p (m c) -> p m c", c=P))

            disw_sb = cp.tile([P, NBLK], F32)
            nc.sync.dma_start(out=disw_sb[:], in_=t_disw[:, :])
            disw2_sb = cp.tile([P, NBLK], F32)
            nc.sync.dma_start(out=disw2_sb[:], in_=t_disw2[:, :])
            idis_f = cp.tile([1, NBLK * P], F32)
            nc.sync.dma_start(out=idis_f[:], in_=t_idisw[:, :])
            idis_bf = cp.tile([1, NBLK * P], BF16)
            nc.vector.tensor_copy(out=idis_bf[:], in_=idis_f[:])

            ident_bf = cp.tile([P, P], BF16)
            make_identity(nc, ident_bf[:])

            w1_f = cp.tile([IN_CH, HID_CH], F32)
            nc.sync.dma_start(out=w1_f[:], in_=t_w1[:, :])
            w1_bf = cp.tile([IN_CH, HID_CH], BF16)
            nc.vector.tensor_copy(out=w1_bf[:], in_=w1_f[:])
            w2_f = cp.tile([HID_CH, OUT_CH], F32)
            nc.sync.dma_start(out=w2_f[:], in_=t_w2[:, :])
            w2_bf = cp.tile([HID_CH, OUT_CH], BF16)
            nc.vector.tensor_copy(out=w2_bf[:], in_=w2_f[:])
            b1_f = cp.tile([1, HID_CH], F32)
            nc.sync.dma_start(out=b1_f[:], in_=t_b1[:, :])
            b1_bf = cp.tile([1, HID_CH], BF16)
            nc.vector.tensor_copy(out=b1_bf[:], in_=b1_f[:])
            b2_f = cp.tile([1, OUT_CH], F32)
            nc.sync.dma_start(out=b2_f[:], in_=t_b2[:, :])
            b2_bf = cp.tile([1, OUT_CH], BF16)
            nc.vector.tensor_copy(out=b2_bf[:], in_=b2_f[:])

            x1stage = cp.tile([P, NBLK, IN_CH], BF16)
            x2stage = cp.tile([P, NBLK, HID_CH], BF16)

            def layer(table, stage, w_bf, brow, oc, emit):
                tabv = table.ap().rearrange("q (r c) -> (q r) c", c=HID_CH)
                half_ap = [tabv[0:VLO, :], tabv[VLO:VTAB, :]]
                for sbx in range(NSB):
                    g = [None, None]
                    for h in (0, 1):
                        Tn = int(T_sec[sbx, h])
                        if Tn == 0:
                            continue
                        T0 = int(TB[sbx, h])
                        gt = gp.tile([P, TMAX[h], HID_CH], BF16, tag=f"g{h}")
                        c0 = 0
                        for cn in _chunks(Tn, gcap):
                            nc.gpsimd.dma_gather(
                                out_ap=gt[:, c0:c0 + cn, :], in_ap=half_ap[h],
                                idxs_ap=idx_sb[:, 8 * (T0 + c0):
                                               8 * (T0 + c0 + cn)],
                                num_idxs=cn * P, num_idxs_reg=cn * P,
                                elem_size=HID_CH,
                                queue_num=(sbx * 2 + h) % 4,
                            )
                            c0 += cn
                        g[h] = gt
                    for bix in range(SBW):
                        b = sbx * SBW + bix
                        oht = [None, None]
                        for h in (0, 1):
                            n = int(nt[sbx, h, bix])
                            if n == 0:
                                continue
                            e0 = int(ebase[sbx, h, bix])
                            ot = op.tile([P, MAXNT, P], BF16, tag=f"oh{h}")
                            nc.vector.tensor_tensor(
                                out=ot[:, :n, :], in0=iota_sb[:, :n, :],
                                in1=escb_sb[:, e0:e0 + n][:, :, None]
                                .to_broadcast([P, n, P]),
                                op=mybir.AluOpType.is_equal,
                            )
                            oht[h] = ot
                        tps = ps.tile([HID_CH, P], F32, tag="tps")
                        first = True
                        for h in (0, 1):
                            n = int(nt[sbx, h, bix])
                            if n == 0:
                                continue
                            f0 = int(ft[sbx, h, bix])
                            for j in range(n):
                                nc.tensor.matmul(
                                    out=tps[:], lhsT=g[h][:, f0 + j, :],
                                    rhs=oht[h][:, j, :],
                                    start=first, stop=False,
                                )
                                first = False
                        # self-loop: tps[ch, v] += stage[v, ch]
                        nc.tensor.matmul(
                            out=tps[:], lhsT=stage[:, b, :], rhs=ident_bf[:],
                            start=first, stop=True,
                        )
                        t_sb = sbp.tile([HID_CH, P], BF16, tag="tsb")
                        nc.scalar.copy(out=t_sb[:], in_=tps[:])
                        ups = ps.tile([P, oc], F32, tag=f"ups{oc}")
                        if no_bias:
                            nc.tensor.matmul(out=ups[:], lhsT=t_sb[:],
                                             rhs=w_bf[:], start=True,
                                             stop=True)
                        else:
                            nc.tensor.matmul(out=ups[:], lhsT=t_sb[:],
                                             rhs=w_bf[:], start=True,
                                             stop=False)
                            nc.tensor.matmul(
                                out=ups[:],
                                lhsT=idis_bf[:, b * P:(b + 1) * P],
                                rhs=brow[:], start=False, stop=True,
                            )
                        emit(b, ups)

            for _rep in range(repeats):
                # ---- P0: build table1 = dis * x (bf16) ----
                for b in range(NBLK):
                    nb = P if b < NFULL else NTAIL
                    xt = sbp.tile([P, IN_CH], F32, tag="xt")
                    if nb < P:
                        nc.vector.memset(xt[:], 0.0)
                    nc.sync.dma_start(out=xt[:nb, :],
                                      in_=t_xsh[b * P: b * P + nb, :])
                    nc.scalar.activation(
                        out=x1stage[:, b, :], in_=xt[:],
                        func=mybir.ActivationFunctionType.Copy,
                        scale=disw_sb[:, b:b + 1],
                    )
                nc.sync.dma_start(
                    out=x1_shard[:, :],
                    in_=x1stage[:].rearrange("p b c -> p (b c)"))
                nc.gpsimd.collective_compute(
                    "AllGather", mybir.AluOpType.bypass, replica_groups=rg,
                    ins=[x1_shard.ap().opt()], outs=[x1_full.ap().opt()],
                )

                # ---- L1 ----
                def emit1(b, ups):
                    nc.scalar.activation(
                        out=x2stage[:, b, :], in_=ups[:],
                        func=mybir.ActivationFunctionType.Relu,
                        scale=disw2_sb[:, b:b + 1],
                    )

                layer(x1_full, x1stage, w1_bf, b1_bf, HID_CH, emit1)

                nc.sync.dma_start(
                    out=x2_shard[:, :],
                    in_=x2stage[:].rearrange("p b c -> p (b c)"))
                nc.gpsimd.collective_compute(
                    "AllGather", mybir.AluOpType.bypass, replica_groups=rg,
                    ins=[x2_shard.ap().opt()], outs=[x2_full.ap().opt()],
                )

                # ---- L2 ----
                def emit2(b, ups):
                    nb = P if b < NFULL else NTAIL
                    ot = sbp.tile([P, OUT_CH], F32, tag="ot")
                    nc.scalar.activation(
                        out=ot[:], in_=ups[:],
                        func=mybir.ActivationFunctionType.Relu,
                        scale=disw_sb[:, b:b + 1],
                    )
                    nc.sync.dma_start(out=t_out[b * P: b * P + nb, :],
                                      in_=ot[:nb, :])

                layer(x2_full, x2stage, w2_bf, b2_bf, OUT_CH, emit2)

    nc.compile()
    return nc


def _in_maps(prep, x, W1, b1, W2, b2):
    maps = []
    for k in range(N_CORES):
        maps.append({
            "xsh": np.ascontiguousarray(x[k * NSH:(k + 1) * NSH]),
            "w1": W1, "b1": np.ascontiguousarray(b1.reshape(1, HID_CH)),
            "w2": W2, "b2": np.ascontiguousarray(b2.reshape(1, OUT_CH)),
            "disw": np.ascontiguousarray(prep["disw"][k]),
            "disw2": np.ascontiguousarray(prep["disw2"][k]),
            "idisw": np.ascontiguousarray(prep["idisw"][k]),
            "idxw": np.ascontiguousarray(prep["idxw"][k]),
            "escb": np.ascontiguousarray(prep["escb"][k]),
            "iotar": np.ascontiguousarray(
                prep["iotar"].reshape(P, -1)),
        })
    return maps


def kernel(x, edge_index, W1, b1, W2, b2, _trace=False, _shared=False,
           _gcap=8, _no_bias=False):
    global LAST_RESULT
    x = np.asarray(x, dtype=np.float32)
    edge_index = np.asarray(edge_index, dtype=np.int32)
    W1 = np.asarray(W1, dtype=np.float32)
    b1 = np.asarray(b1, dtype=np.float32)
    W2 = np.asarray(W2, dtype=np.float32)
    b2 = np.asarray(b2, dtype=np.float32)

    prep = _host_prep(edge_index)
    nc = _build(prep, shared_tables=_shared, gcap=_gcap, no_bias=_no_bias)
    maps = _in_maps(prep, x, W1, b1, W2, b2)
    res = run_bass_kernel_spmd(nc, maps, core_ids=list(range(N_CORES)),
                               trace=_trace)
    LAST_RESULT = res
    out = np.concatenate([res.results[k]["out"] for k in range(N_CORES)],
                         axis=0)
    return out.astype(np.float32)


# revision 22
# speedup vs baseline: 1.7646x; 1.7646x over previous
"""2-layer GCN (GCNConv 128->128->64, N=50000, E=800000) on 8 TRN2 NeuronCores.

Strategy (dst-sharded, aggregate-first, v2):
  out = relu(A_hat @ relu(A_hat @ x @ W1 + b1) @ W2 + b2),  A_hat = D^-1/2 (A+I) D^-1/2
  - Gather tables hold dis-scaled features (t1 = dis*x, t2 = dis*relu(...)), so
    one-hot scatter matrices are PURE 0/1 (single DVE is_equal per block-section).
  - dst-side dis folded into the per-block epilogue: weight matmul emitted as
    matmul(lhsT=t_sb, rhs=W) giving [dst, oc] output directly (no transposes);
    activation applies per-partition scale (dis or dis^2) + Relu; bias enters
    via a rank-1 (K=1) PSUM matmul outer(1/dis, b) before the scale.
  - Self-loop term is matmul(lhsT=stage_block, rhs=I) (transposes the block).
  - Edges sorted by (core, superblock of 7 dst-blocks, table-part, dst) and
    packed per (superblock, part): sections pad to 128 only once per section
    (T_total ~ 804 vs 870 per-block-padded). dma_gather calls are capped at
    1024 idxs (SWDGE ucode limit) and rotate the 4 SWDGE queues PER CALL --
    consecutive calls overlap their DMA/sem phases across queues (~2x wall
    win vs per-section rotation). Per-block tile subranges (uniform across
    cores) drive the PSUM-accumulated matmuls; edges of neighboring blocks in
    shared boundary tiles are nulled by the one-hot (esc values are
    block-shifted on host so only own-block edges land in 0..127).
  - Gather table split into part A (28 blocks) / part B (21 blocks), each
    int16-addressable; each part AllGathers separately (contiguous tensors) as
    soon as its stage columns are done, overlapping the collective with the
    producing layer's tail.
Host-side work is index-only prep + output concat.
"""

import numpy as np
import ml_dtypes

import concourse.bass as bass
import concourse.bacc as bacc
import concourse.mybir as mybir
import concourse.tile as tile
from concourse.bass_utils import run_bass_kernel_spmd
from concourse.library_config import mlp
from concourse.masks import make_identity

P = 128
N_NODES = 50000
N_EDGES = 800000
IN_CH = 128
HID_CH = 128
OUT_CH = 64
N_CORES = 8
NSH = N_NODES // N_CORES          # 6250 nodes per core
NBLK = (NSH + P - 1) // P         # 49 blocks per core (48 full + 106 tail)
SBW = 7                           # blocks per superblock
NSB = NBLK // SBW                 # 7 superblocks
NWRAP = NBLK * P                  # 6272 table slots per core-shard
NFULL = NSH // P                  # 48 full blocks
NTAIL = NSH - NFULL * P           # 106
CSPL = 28                         # blocks in table part A (rest in part B)
CSPLB = NBLK - CSPL               # 21 blocks in part B
CSPL_SB = CSPL // SBW - 1         # last superblock fully inside part A
VTA = N_CORES * P * CSPL          # 28672 rows of table A ( < 32768, int16 ok)
VTB = N_CORES * P * CSPLB         # 21504 rows of table B

BF16 = mybir.dt.bfloat16
F16 = mybir.dt.float16
F32 = mybir.dt.float32

LAST_RESULT = None  # for test harness: BassKernelResults of last run


def _host_prep(edge_index):
    """Index-only preprocessing. Returns per-core upload arrays + tile plan."""
    src = edge_index[0].astype(np.int64)
    dst = edge_index[1].astype(np.int64)
    E = src.shape[0]

    deg = np.bincount(dst, minlength=N_NODES) + 1
    dis = 1.0 / np.sqrt(deg.astype(np.float64))
    idis = np.sqrt(deg.astype(np.float64))

    core = dst // NSH
    ic = dst - core * NSH
    blk = ic // P
    sb = blk // SBW
    bi = blk - sb * SBW
    drel = ic - sb * (SBW * P)          # 0..895 superblock-relative dst

    # split table: part A = blocks [0, CSPL), part B = blocks [CSPL, NBLK)
    k = src // NSH
    i = src - k * NSH
    r = i // P
    half = (r >= CSPL).astype(np.int64)
    srow = np.where(half == 0,
                    (k * P + (i % P)) * CSPL + r,
                    (k * P + (i % P)) * CSPLB + (r - CSPL))

    skey = (core * NSB + sb) * 2 + half          # per-core section id
    order = np.lexsort((dst, skey))              # sort by section, then dst
    s_skey = skey[order]
    s_core = core[order]
    s_drel = drel[order]
    s_row = srow[order]

    n_sec = N_CORES * NSB * 2
    cnt = np.bincount(s_skey, minlength=n_sec).reshape(N_CORES, NSB * 2)
    T_sec = np.ceil(cnt.max(axis=0) / P).astype(np.int64)   # [NSB*2]
    TB = np.concatenate([[0], np.cumsum(T_sec)])[:-1]       # [NSB*2]
    T_total = int(T_sec.sum())

    sec_start = np.concatenate([[0], np.cumsum(cnt.reshape(-1))])[:-1]
    pos = np.arange(E) - sec_start[s_skey]
    slot = TB[s_skey % (NSB * 2)] * P + pos      # slot within core's edge list

    EPC = T_total * P
    idx_rows = np.zeros((N_CORES, EPC), np.int64)
    drel_arr = np.full((N_CORES, EPC), -3000, np.int64)
    idx_rows[s_core, slot] = s_row
    drel_arr[s_core, slot] = s_drel

    # per (sb, h, bi) tile subranges, uniform across cores
    key4 = skey * SBW + bi
    cnt_blk = np.bincount(key4, minlength=n_sec * SBW).reshape(
        N_CORES, NSB, 2, SBW)
    s0 = np.cumsum(cnt_blk, axis=3) - cnt_blk            # exclusive start
    e0 = s0 + cnt_blk
    ft_c = s0 // P
    lt_c = -(-e0 // P)
    has = cnt_blk > 0
    BIG = 10 ** 6
    ft = np.where(has, ft_c, BIG).min(axis=0)            # [NSB, 2, SBW]
    lt = np.where(has, lt_c, -1).max(axis=0)
    nt = np.maximum(lt - ft, 0)
    ft = np.where(nt > 0, ft, 0)
    MAXNT = int(nt.max())

    # esc_blk: per-(sb,h,bi) block-shifted dst-rel values, f16-exact ints
    drel_wrap = drel_arr.reshape(N_CORES, T_total, P).transpose(0, 2, 1)
    cols = []
    ebase = np.zeros((NSB, 2, SBW), np.int64)
    run = 0
    for sbx in range(NSB):
        for h in (0, 1):
            for bix in range(SBW):
                n = int(nt[sbx, h, bix])
                if n == 0:
                    ebase[sbx, h, bix] = -1
                    continue
                t0 = int(TB[sbx * 2 + h] + ft[sbx, h, bix])
                c = drel_wrap[:, :, t0:t0 + n] - 128 * bix
                cols.append(np.clip(c, -2000, 2000))
                ebase[sbx, h, bix] = run
                run += n
    escb = np.concatenate(cols, axis=2).astype(np.float16)  # [NC, P, NTT]
    NTT = run

    # wrap indices: idx i -> [i%16, i//16], replicated to 128 partitions
    ii = np.arange(EPC)
    w = np.zeros((N_CORES, 16, T_total * 8), np.int16)
    w[:, ii % 16, ii // 16] = idx_rows
    idxw = np.tile(w, (1, 8, 1))                 # [NC, 128, T*8]

    # disw / disw2 [NC, P, NBLK]; idisw [NC, 1, NBLK*P]
    nodes = np.arange(NBLK * P)
    valid = nodes < NSH
    disw = np.zeros((N_CORES, P, NBLK), np.float32)
    idisw = np.zeros((N_CORES, 1, NBLK * P), np.float32)
    for c in range(N_CORES):
        v = np.zeros(NBLK * P)
        v[valid] = dis[c * NSH + nodes[valid]]
        disw[c] = v.reshape(NBLK, P).T.astype(np.float32)
        u = np.zeros(NBLK * P)
        u[valid] = idis[c * NSH + nodes[valid]]
        idisw[c, 0] = u.astype(np.float32)
    disw2 = (disw.astype(np.float64) ** 2).astype(np.float32)

    iotar = np.tile(np.arange(P, dtype=np.float16), (P, max(MAXNT, 1), 1))

    return {
        "T_sec": T_sec.reshape(NSB, 2), "TB": TB.reshape(NSB, 2),
        "ft": ft, "nt": nt, "ebase": ebase, "T_total": T_total,
        "MAXNT": MAXNT, "NTT": NTT,
        "idxw": idxw.astype(np.int16), "escb": escb,
        "disw": disw, "disw2": disw2, "idisw": idisw, "iotar": iotar,
    }


def _chunks(t, cap):
    """Split t tiles into balanced chunks of <= cap (9 -> 5+4, not 8+1)."""
    if t == 0:
        return []
    if cap is None or t <= cap:
        return [t]
    n = -(-t // cap)
    base, rem = divmod(t, n)
    return [base + (1 if i < rem else 0) for i in range(n)]


def _build(prep, repeats=1, shared_tables=False, gcap=8, no_bias=False,
           skip_coll=False, skip_gather=False, skip_agg=False,
           prep_trigger=False):
    T_sec, TB = prep["T_sec"], prep["TB"]
    ft, nt, ebase = prep["ft"], prep["nt"], prep["ebase"]
    T_total, MAXNT, NTT = prep["T_total"], prep["MAXNT"], prep["NTT"]
    TMAX = [int(T_sec[:, 0].max()), int(T_sec[:, 1].max())]

    nc = bacc.Bacc("TRN2", target_bir_lowering=False, num_devices=N_CORES,
                   num_swdge_queues=4)

    t_xsh = nc.dram_tensor("xsh", [NSH, IN_CH], F32, kind="ExternalInput")
    t_w1 = nc.dram_tensor("w1", [IN_CH, HID_CH], F32, kind="ExternalInput")
    t_b1 = nc.dram_tensor("b1", [1, HID_CH], F32, kind="ExternalInput")
    t_w2 = nc.dram_tensor("w2", [HID_CH, OUT_CH], F32, kind="ExternalInput")
    t_b2 = nc.dram_tensor("b2", [1, OUT_CH], F32, kind="ExternalInput")
    t_disw = nc.dram_tensor("disw", [P, NBLK], F32, kind="ExternalInput")
    t_disw2 = nc.dram_tensor("disw2", [P, NBLK], F32, kind="ExternalInput")
    t_idisw = nc.dram_tensor("idisw", [1, NBLK * P], F32, kind="ExternalInput")
    t_idxw = nc.dram_tensor("idxw", [P, T_total * 8], mybir.dt.int16,
                            kind="ExternalInput")
    t_escb = nc.dram_tensor("escb", [P, NTT], F16, kind="ExternalInput")
    t_iotar = nc.dram_tensor("iotar", [P, max(MAXNT, 1) * P], F16,
                             kind="ExternalInput")
    t_out = nc.dram_tensor("out", [NSH, OUT_CH], F32, kind="ExternalOutput")

    aspace = "Shared" if shared_tables else "Local"
    NWA, NWB = CSPL * HID_CH, CSPLB * HID_CH
    x1_shardA = nc.dram_tensor("x1_shardA", [P, NWA], BF16)
    x1_shardB = nc.dram_tensor("x1_shardB", [P, NWB], BF16)
    x1_fullA = nc.dram_tensor("x1_fullA", [N_CORES * P, NWA], BF16,
                              addr_space=aspace)
    x1_fullB = nc.dram_tensor("x1_fullB", [N_CORES * P, NWB], BF16,
                              addr_space=aspace)
    x2_shardA = nc.dram_tensor("x2_shardA", [P, NWA], BF16)
    x2_shardB = nc.dram_tensor("x2_shardB", [P, NWB], BF16)
    x2_fullA = nc.dram_tensor("x2_fullA", [N_CORES * P, NWA], BF16,
                              addr_space=aspace)
    x2_fullB = nc.dram_tensor("x2_fullB", [N_CORES * P, NWB], BF16,
                              addr_space=aspace)

    rg = [list(range(N_CORES))]
    gsems = [nc.alloc_semaphore(f"gsem{q}") for q in range(4)]
    gcnt = [0, 0, 0, 0]
    qrr = [0]

    with tile.TileContext(nc) as tc:
        with (
            tc.tile_pool(name="const", bufs=1) as cp,
            tc.tile_pool(name="gpool", bufs=2) as gp,
            tc.tile_pool(name="opool", bufs=4) as op,
            tc.tile_pool(name="sbuf", bufs=3) as sbp,
            tc.tile_pool(name="psum", bufs=3, space="PSUM") as ps,
            tc.tile_pool(name="psumu", bufs=2, space="PSUM") as psu,
        ):
            nc.gpsimd.load_library(mlp)

            idx_sb = cp.tile([P, T_total * 8], mybir.dt.int16)
            nc.sync.dma_start(out=idx_sb[:], in_=t_idxw[:, :])
            escb_sb = cp.tile([P, NTT], F16)
            nc.sync.dma_start(out=escb_sb[:], in_=t_escb[:, :])
            iota_sb = cp.tile([P, max(MAXNT, 1), P], F16)
            nc.sync.dma_start(
                out=iota_sb[:],
                in_=t_iotar[:, :].rearrange("p (m c) -> p m c", c=P))

            disw_sb = cp.tile([P, NBLK], F32)
            nc.sync.dma_start(out=disw_sb[:], in_=t_disw[:, :])
            disw2_sb = cp.tile([P, NBLK], F32)
            nc.sync.dma_start(out=disw2_sb[:], in_=t_disw2[:, :])
            idis_f = cp.tile([1, NBLK * P], F32)
            nc.sync.dma_start(out=idis_f[:], in_=t_idisw[:, :])
            idis_bf = cp.tile([1, NBLK * P], BF16)
            nc.vector.tensor_copy(out=idis_bf[:], in_=idis_f[:])

            ident_bf = cp.tile([P, P], BF16)
            make_identity(nc, ident_bf[:])

            w1_f = cp.tile([IN_CH, HID_CH], F32)
            nc.sync.dma_start(out=w1_f[:], in_=t_w1[:, :])
            w1_bf = cp.tile([IN_CH, HID_CH], BF16)
            nc.vector.tensor_copy(out=w1_bf[:], in_=w1_f[:])
            w2_f = cp.tile([HID_CH, OUT_CH], F32)
            nc.sync.dma_start(out=w2_f[:], in_=t_w2[:, :])
            w2_bf = cp.tile([HID_CH, OUT_CH], BF16)
            nc.vector.tensor_copy(out=w2_bf[:], in_=w2_f[:])
            b1_f = cp.tile([1, HID_CH], F32)
            nc.sync.dma_start(out=b1_f[:], in_=t_b1[:, :])
            b1_bf = cp.tile([1, HID_CH], BF16)
            nc.vector.tensor_copy(out=b1_bf[:], in_=b1_f[:])
            b2_f = cp.tile([1, OUT_CH], F32)
            nc.sync.dma_start(out=b2_f[:], in_=t_b2[:, :])
            b2_bf = cp.tile([1, OUT_CH], BF16)
            nc.vector.tensor_copy(out=b2_bf[:], in_=b2_f[:])

            x1stage = cp.tile([P, NBLK, IN_CH], BF16)
            x2stage = cp.tile([P, NBLK, HID_CH], BF16)

            def layer(tables, stage, w_bf, brow, oc, emit, post_block=None):
                half_ap = [
                    tables[0].ap().rearrange("q (r c) -> (q r) c", c=HID_CH),
                    tables[1].ap().rearrange("q (r c) -> (q r) c", c=HID_CH),
                ]
                for sbx in range(NSB):
                    g = [None, None]
                    gwait = [None, None]
                    for h in (0, 1):
                        Tn = int(T_sec[sbx, h])
                        if Tn == 0:
                            continue
                        T0 = int(TB[sbx, h])
                        gt = gp.tile([P, TMAX[h], HID_CH], BF16, tag=f"g{h}")
                        c0 = 0
                        chunk_list = (_chunks(Tn, gcap) if not skip_gather
                                      else [])
                        for ci, cn in enumerate(chunk_list):
                            q = qrr[0] % 4
                            qrr[0] += 1
                            nc.gpsimd.dma_gather(
                                out_ap=gt[:, c0:c0 + cn, :], in_ap=half_ap[h],
                                idxs_ap=idx_sb[:, 8 * (T0 + c0):
                                               8 * (T0 + c0 + cn)],
                                num_idxs=cn * P, num_idxs_reg=cn * P,
                                elem_size=HID_CH, queue_num=q,
                                **({"prepare_only": True, "sem": gsems[q]}
                                   if prep_trigger else {}),
                            )
                            c0 += cn
                            if prep_trigger:
                                gcnt[q] += 1
                                nc.gpsimd.trigger_dma(count=None, queue_num=q)
                        g[h] = gt
                        if prep_trigger:
                            gwait[h] = (gsems[q], 16 * gcnt[q])
                    for bix in range(SBW):
                        b = sbx * SBW + bix
                        oht = [None, None]
                        for h in ((0, 1) if not skip_agg else ()):
                            n = int(nt[sbx, h, bix])
                            if n == 0:
                                continue
                            e0 = int(ebase[sbx, h, bix])
                            ot = op.tile([P, MAXNT, P], BF16, tag=f"oh{h}")
                            nc.vector.tensor_tensor(
                                out=ot[:, :n, :], in0=iota_sb[:, :n, :],
                                in1=escb_sb[:, e0:e0 + n][:, :, None]
                                .to_broadcast([P, n, P]),
                                op=mybir.AluOpType.is_equal,
                            )
                            oht[h] = ot
                        tps = ps.tile([HID_CH, P], F32, tag="tps")
                        first = True
                        for h in ((0, 1) if not skip_agg else ()):
                            n = int(nt[sbx, h, bix])
                            if n == 0:
                                continue
                            f0 = int(ft[sbx, h, bix])
                            for j in range(n):
                                if j == 0 and gwait[h] is not None:
                                    nc.tensor.wait_ge(gwait[h][0],
                                                      gwait[h][1])
                                nc.tensor.matmul(
                                    out=tps[:], lhsT=g[h][:, f0 + j, :],
                                    rhs=oht[h][:, j, :],
                                    start=first, stop=False,
                                )
                                first = False
                        # self-loop: tps[ch, v] += stage[v, ch]
                        nc.tensor.matmul(
                            out=tps[:], lhsT=stage[:, b, :], rhs=ident_bf[:],
                            start=first, stop=True,
                        )
                        t_sb = sbp.tile([HID_CH, P], BF16, tag="tsb")
                        nc.scalar.copy(out=t_sb[:], in_=tps[:])
                        ups = psu.tile([P, oc], F32, tag=f"ups{oc}")
                        if no_bias:
                            nc.tensor.matmul(out=ups[:], lhsT=t_sb[:],
                                             rhs=w_bf[:], start=True,
                                             stop=True)
                        else:
                            nc.tensor.matmul(out=ups[:], lhsT=t_sb[:],
                                             rhs=w_bf[:], start=True,
                                             stop=False)
                            nc.tensor.matmul(
                                out=ups[:],
                                lhsT=idis_bf[:, b * P:(b + 1) * P],
                                rhs=brow[:], start=False, stop=True,
                            )
                        emit(b, ups)
                        if post_block is not None:
                            post_block(b)

            for _rep in range(repeats):
                # ---- P0: build table1 = dis * x (bf16) ----
                for sbx in range(NSB):
                    b0 = sbx * SBW
                    nfb = SBW if sbx < NSB - 1 else (NFULL - b0)
                    xt = sbp.tile([P, SBW, IN_CH], F32, tag="xt")
                    nc.sync.dma_start(
                        out=xt[:, :nfb, :],
                        in_=t_xsh[b0 * P: (b0 + nfb) * P, :]
                        .rearrange("(b p) c -> p b c", p=P))
                    if sbx == NSB - 1:
                        nc.vector.memset(xt[:, nfb, :], 0.0)
                        nc.sync.dma_start(
                            out=xt[:NTAIL, nfb, :],
                            in_=t_xsh[NFULL * P:, :])
                    for bi in range(SBW):
                        b = b0 + bi
                        nc.scalar.activation(
                            out=x1stage[:, b, :], in_=xt[:, bi, :],
                            func=mybir.ActivationFunctionType.Copy,
                            scale=disw_sb[:, b:b + 1],
                        )
                    if sbx == CSPL_SB and not skip_coll:
                        nc.sync.dma_start(
                            out=x1_shardA[:, :],
                            in_=x1stage[:, :CSPL, :]
                            .rearrange("p b c -> p (b c)"))
                        nc.gpsimd.collective_compute(
                            "AllGather", mybir.AluOpType.bypass,
                            replica_groups=rg,
                            ins=[x1_shardA.ap().opt()],
                            outs=[x1_fullA.ap().opt()],
                        )
                nc.sync.dma_start(
                    out=x1_shardB[:, :],
                    in_=x1stage[:, CSPL:, :].rearrange("p b c -> p (b c)"))
                if not skip_coll:
                    nc.gpsimd.collective_compute(
                        "AllGather", mybir.AluOpType.bypass, replica_groups=rg,
                        ins=[x1_shardB.ap().opt()], outs=[x1_fullB.ap().opt()],
                    )

                # ---- L1 ----
                def emit1(b, ups):
                    nc.scalar.activation(
                        out=x2stage[:, b, :], in_=ups[:],
                        func=mybir.ActivationFunctionType.Relu,
                        scale=disw2_sb[:, b:b + 1],
                    )

                def post1(b):
                    if b == CSPL - 1 and not skip_coll:
                        nc.sync.dma_start(
                            out=x2_shardA[:, :],
                            in_=x2stage[:, :CSPL, :]
                            .rearrange("p b c -> p (b c)"))
                        nc.gpsimd.collective_compute(
                            "AllGather", mybir.AluOpType.bypass,
                            replica_groups=rg,
                            ins=[x2_shardA.ap().opt()],
                            outs=[x2_fullA.ap().opt()],
                        )

                layer((x1_fullA, x1_fullB), x1stage, w1_bf, b1_bf, HID_CH,
                      emit1, post_block=post1)

                nc.sync.dma_start(
                    out=x2_shardB[:, :],
                    in_=x2stage[:, CSPL:, :].rearrange("p b c -> p (b c)"))
                if not skip_coll:
                    nc.gpsimd.collective_compute(
                        "AllGather", mybir.AluOpType.bypass, replica_groups=rg,
                        ins=[x2_shardB.ap().opt()], outs=[x2_fullB.ap().opt()],
                    )

                # ---- L2 ----
                def emit2(b, ups):
                    nb = P if b < NFULL else NTAIL
                    ot = sbp.tile([P, OUT_CH], F32, tag="ot")
                    nc.scalar.activation(
                        out=ot[:], in_=ups[:],
                        func=mybir.ActivationFunctionType.Relu,
                        scale=disw_sb[:, b:b + 1],
                    )
                    nc.sync.dma_start(out=t_out[b * P: b * P + nb, :],
                                      in_=ot[:nb, :])

                layer((x2_fullA, x2_fullB), x2stage, w2_bf, b2_bf, OUT_CH,
                      emit2)

    nc.compile()
    return nc


def _in_maps(prep, x, W1, b1, W2, b2):
    maps = []
    for k in range(N_CORES):
        maps.append({
            "xsh": np.ascontiguousarray(x[k * NSH:(k + 1) * NSH]),
            "w1": W1, "b1": np.ascontiguousarray(b1.reshape(1, HID_CH)),
            "w2": W2, "b2": np.ascontiguousarray(b2.reshape(1, OUT_CH)),
            "disw": np.ascontiguousarray(prep["disw"][k]),
            "disw2": np.ascontiguousarray(prep["disw2"][k]),
            "idisw": np.ascontiguousarray(prep["idisw"][k]),
            "idxw": np.ascontiguousarray(prep["idxw"][k]),
            "escb": np.ascontiguousarray(prep["escb"][k]),
            "iotar": np.ascontiguousarray(
                prep["iotar"].reshape(P, -1)),
        })
    return maps


def kernel(x, edge_index, W1, b1, W2, b2, _trace=False, _shared=False,
           _gcap=8, _no_bias=False):
    global LAST_RESULT
    x = np.asarray(x, dtype=np.float32)
    edge_index = np.asarray(edge_index, dtype=np.int32)
    W1 = np.asarray(W1, dtype=np.float32)
    b1 = np.asarray(b1, dtype=np.float32)
    W2 = np.asarray(W2, dtype=np.float32)
    b2 = np.asarray(b2, dtype=np.float32)

    prep = _host_prep(edge_index)
    nc = _build(prep, shared_tables=_shared, gcap=_gcap, no_bias=_no_bias)
    maps = _in_maps(prep, x, W1, b1, W2, b2)
    res = run_bass_kernel_spmd(nc, maps, core_ids=list(range(N_CORES)),
                               trace=_trace)
    LAST_RESULT = res
    out = np.concatenate([res.results[k]["out"] for k in range(N_CORES)],
                         axis=0)
    return out.astype(np.float32)
